# revision 1
# baseline (speedup 1.0000x reference)
"""ABCNN-2 matching block on 8 TRN2 NeuronCores (pure data parallel).

Math (per batch b):
    x0 = F0r[b] (mask is all-ones -> skipped), x1 = F1r[b]
    c = tanh(conv1d_full(x, w) + bias)            # [L'=515, D=512]
    A[i,j] = 1 / (1 + ||c0_i - c1_j||)            # via GEMM expansion
    a0 = rowsum(A), a1 = colsum(A)
    out0 = avgpool4(c0 * a0), out1 = avgpool4(c1 * a1)   # [512, D]

Device layout: c kept as [d-part, i-free] bf16. Conv is done on the PE with
two static band matrices; d2 is assembled entirely in PSUM (K=2 init matmul
with -0.5*sq rows + bf16 cross GEMM), epilogue is ACT sqrt + DVE recip.
Outputs are written [b, d, t]; the host transposes to [b, t, d].
"""

import numpy as np

B, L, D, W = 64, 512, 512, 4
LP = L + W - 1  # 515
NCORES = 8
BL = B // NCORES  # 8 batches per core
P = 128
NDC = D // P   # 4 d-chunks
NTC = L // P   # 4 t-chunks
M_CHUNKS = [(m * P, min(P, LP - m * P)) for m in range((LP + P - 1) // P)]  # 5

_CACHE = {}
ABL = "full"  # ablation knob for perf debugging: full|p1|nogemm|noepi|noout
NREP_DEV = 1  # replicate the whole computation inside the NEFF (bench only)


def _build_consts(conv_w, conv_b):
    import ml_dtypes

    w = np.asarray(conv_w, np.float32).reshape(W)  # OIHW [1,1,4,1] -> [4]
    # y[d, i] = sum_t x[t, d] * Wband[t, i],  Wband[t, i] = w[t - i + 3]
    # main block: t,i in same 128-chunk; prev block: t in chunk m-1, i in chunk m
    r = np.arange(P)[:, None]
    c = np.arange(P)[None, :]
    k_main = r - c + 3
    wmain = np.where((k_main >= 0) & (k_main < W), w[np.clip(k_main, 0, 3)], 0.0)
    k_prev = r - c - 125
    wprev = np.where((k_prev >= 0) & (k_prev < W), w[np.clip(k_prev, 0, 3)], 0.0)
    return {
        "Wmain": wmain.astype(ml_dtypes.bfloat16),
        "Wprev": wprev.astype(ml_dtypes.bfloat16),
        "ones_col": np.ones((P, 1), ml_dtypes.bfloat16),
        "eye": np.eye(P, dtype=np.float32),
        "convb": np.full((P, 1), np.float32(np.asarray(conv_b).reshape(-1)[0])),
    }


def _build_bass():
    import concourse.bass as bass
    import concourse.tile as tile
    from concourse import bacc, mybir

    DT = mybir.dt
    AF = mybir.ActivationFunctionType
    OP = mybir.AluOpType

    nc = bacc.Bacc("TRN2", target_bir_lowering=False, debug=False,
                   num_devices=NCORES)

    F = [nc.dram_tensor(f"F{s}", [BL, L, D], DT.float32, kind="ExternalInput")
         for s in range(2)]
    wmain_d = nc.dram_tensor("Wmain", [P, P], DT.bfloat16, kind="ExternalInput")
    wprev_d = nc.dram_tensor("Wprev", [P, P], DT.bfloat16, kind="ExternalInput")
    ones_d = nc.dram_tensor("ones_col", [P, 1], DT.bfloat16, kind="ExternalInput")
    eye_d = nc.dram_tensor("eye", [P, P], DT.float32, kind="ExternalInput")
    convb_d = nc.dram_tensor("convb", [P, 1], DT.float32, kind="ExternalInput")
    O = [nc.dram_tensor(f"o{s}", [BL, D, L], DT.float32, kind="ExternalOutput")
         for s in range(2)]

    with tile.TileContext(nc) as tc:
        from contextlib import ExitStack

        with ExitStack() as ctx:
            statics = ctx.enter_context(tc.tile_pool(name="statics", bufs=1))
            wmain_t = statics.tile([P, P], DT.bfloat16)
            nc.sync.dma_start(wmain_t[:], wmain_d.ap())
            wprev_t = statics.tile([P, P], DT.bfloat16)
            nc.sync.dma_start(wprev_t[:], wprev_d.ap())
            ones_t = statics.tile([P, 1], DT.bfloat16)
            nc.sync.dma_start(ones_t[:], ones_d.ap())
            eye_t = statics.tile([P, P], DT.float32)
            nc.sync.dma_start(eye_t[:], eye_d.ap())
            convb_t = statics.tile([P, 1], DT.float32)
            nc.sync.dma_start(convb_t[:], convb_d.ap())
            eps_t = statics.tile([P, 1], DT.float32)
            nc.vector.memset(eps_t[:], 1e-6)

            # c tiles and sq-row tiles live across both phases
            cpool = ctx.enter_context(tc.tile_pool(name="c", bufs=2 * BL))
            rowsp = ctx.enter_context(tc.tile_pool(name="sqrows", bufs=2 * BL))
            c_tiles = [[None, None] for _ in range(BL)]
            row_tiles = [[None, None] for _ in range(BL)]
            dumo = ctx.enter_context(tc.tile_pool(name="dumo", bufs=2))

            def _dummy_out(b, s, src):
                ot = dumo.tile([P, NDC, L], DT.float32)
                nc.vector.tensor_copy(ot[:, 0, 0:512], src)
                nc.sync.dma_start(
                    O[s].ap()[b].rearrange("(c p) t -> p c t", p=P), ot[:])

            for _rep in range(NREP_DEV):
                # ---------------- Phase 1: conv + tanh + squared norms ----------
                with ExitStack() as p1:
                    yp = p1.enter_context(tc.tile_pool(name="psy", bufs=3, space="PSUM"))
                    sqp = p1.enter_context(tc.tile_pool(name="pssq", bufs=1, space="PSUM"))
                    xfp = p1.enter_context(tc.tile_pool(name="xf", bufs=2))
                    xbp = p1.enter_context(tc.tile_pool(name="xb", bufs=2))
                    csqp = p1.enter_context(tc.tile_pool(name="csq", bufs=2))
                    ysbp = p1.enter_context(tc.tile_pool(name="ysb", bufs=2))

                    for b in range(BL):
                        for s in range(2):
                            xf = xfp.tile([P, NTC, L], DT.float32)
                            xsrc = F[s].ap()[b].rearrange("(c p) d -> c p d", p=P)
                            for tcn in range(NTC):
                                nc.sync.dma_start(xf[:, tcn, :], xsrc[tcn])
                            if ABL == "dma":
                                _dummy_out(b, s, xf[:, 0, 0:512])
                                continue
                            xb = xbp.tile([P, NTC, L], DT.bfloat16)
                            nc.vector.tensor_copy(xb[:], xf[:])
                            if ABL == "xconv":
                                _dummy_out(b, s, xb[:, 0, 0:512])
                                continue

                            c_t = cpool.tile([P, NDC * 520], DT.bfloat16)
                            c_tiles[b][s] = c_t
                            ysb = ysbp.tile([P, NDC * 520], DT.float32)
                            for dc in range(NDC):
                                nc.vector.memset(
                                    ysb[:, 520 * dc + LP: 520 * dc + 520], 0.0)
                            for dc in range(NDC):
                                y = yp.tile([P, LP], DT.float32)
                                xcol = slice(dc * P, dc * P + P)
                                # bank0 (cols 0..511): one start (first MM) and
                                # one stop (last MM); bank1 (512..514) its own.
                                for ic in range(4):
                                    nc.tensor.matmul(
                                        y[:, ic * P: ic * P + P],
                                        lhsT=xb[:, ic, xcol], rhs=wmain_t[:, 0:P],
                                        start=(ic == 0), stop=False)
                                for ic in range(1, 4):
                                    nc.tensor.matmul(
                                        y[:, ic * P: ic * P + 3],
                                        lhsT=xb[:, ic - 1, xcol], rhs=wprev_t[:, 0:3],
                                        start=False, stop=(ic == 3))
                                nc.tensor.matmul(
                                    y[:, 512:515],
                                    lhsT=xb[:, 3, xcol], rhs=wprev_t[:, 0:3],
                                    start=True, stop=True)
                                # drain psum fast on DVE; ACT works from SBUF
                                nc.vector.tensor_copy(
                                    ysb[:, 520 * dc: 520 * dc + LP], y[:, 0:LP])
                            # one big tanh per (b, s); gap columns hold garbage
                            # but tanh is bounded and gaps are never read
                            nc.scalar.activation(
                                c_t[:, 0:NDC * 520], ysb[:, 0:NDC * 520],
                                AF.Tanh, bias=convb_t[:, 0:1], scale=1.0)

                            if ABL == "conv":
                                _dummy_out(b, s, c_t[:, 0:512])
                                continue
                            # squared norms: sq[i] = sum_d c[d,i]^2
                            csq = csqp.tile([P, NDC * 520], DT.bfloat16)
                            for dc in range(NDC):
                                nc.gpsimd.tensor_mul(
                                    csq[:, 520 * dc: 520 * dc + LP],
                                    c_t[:, 520 * dc: 520 * dc + LP],
                                    c_t[:, 520 * dc: 520 * dc + LP])
                            sq = sqp.tile([1, LP], DT.float32)
                            for kc in range(NDC):
                                nc.tensor.matmul(
                                    sq[0:1, 0:512], lhsT=ones_t[:, 0:1],
                                    rhs=csq[:, 520 * kc: 520 * kc + 512],
                                    start=(kc == 0), stop=(kc == NDC - 1))
                                nc.tensor.matmul(
                                    sq[0:1, 512:515], lhsT=ones_t[:, 0:1],
                                    rhs=csq[:, 520 * kc + 512: 520 * kc + 515],
                                    start=(kc == 0), stop=(kc == NDC - 1))
                            # rows for the K=2 init matmul:
                            #   sqlhs (s=0): p0 = -0.5*sq0, p1 = 1
                            #   sqrhs (s=1): p0 = 1,        p1 = -0.5*sq1
                            # compute engines can't address base-partition 1, so
                            # the p1 data row is staged and DMA'd into place.
                            row = rowsp.tile([2, 520], DT.float32)
                            row_tiles[b][s] = row
                            nc.vector.memset(row[0:2, 0:520], 1.0)
                            if s == 0:
                                nc.vector.tensor_scalar(
                                    out=row[0:1, 0:LP], in0=sq[0:1, 0:LP],
                                    scalar1=-0.5, scalar2=None, op0=OP.mult)
                            else:
                                tmp = rowsp.tile([1, 520], DT.float32, tag="rowtmp")
                                nc.vector.tensor_scalar(
                                    out=tmp[0:1, 0:LP], in0=sq[0:1, 0:LP],
                                    scalar1=-0.5, scalar2=None, op0=OP.mult)
                                nc.sync.dma_start(row[1:2, 0:LP], tmp[0:1, 0:LP])

                # ---------------- Phase 2: GEMM + match epilogue + outputs ------
                if ABL == "p1":
                    for b in range(BL):
                        for s in range(2):
                            _dummy_out(b, s, c_tiles[b][s][:, 0:512])
                with ExitStack() as p2:
                    Ap = p2.enter_context(tc.tile_pool(name="psA", bufs=2, space="PSUM"))
                    rp = p2.enter_context(tc.tile_pool(name="psrow", bufs=2, space="PSUM"))
                    distp = p2.enter_context(tc.tile_pool(name="dist", bufs=2))
                    afp = p2.enter_context(tc.tile_pool(name="af32", bufs=2))
                    abfp = p2.enter_context(tc.tile_pool(name="abf", bufs=2))
                    a0p = p2.enter_context(tc.tile_pool(name="a0c", bufs=2))
                    arp = p2.enter_context(tc.tile_pool(name="arow", bufs=4))
                    abp = p2.enter_context(tc.tile_pool(name="abcast", bufs=4))
                    gp = p2.enter_context(tc.tile_pool(name="g", bufs=2))
                    s1p = p2.enter_context(tc.tile_pool(name="s1", bufs=2))
                    otp = p2.enter_context(tc.tile_pool(name="ot", bufs=1))

                    for b in (range(BL) if ABL in ("full", "nogemm", "noepi", "noout")
                              else []):
                        c0_t, c1_t = c_tiles[b]
                        sqlhs, sqrhs = row_tiles[b]
                        a0cols = a0p.tile([P, 8], DT.float32)
                        ssb = distp.tile([P, 5 * 520], DT.float32, tag="ssb")
                        for m in range(5):
                            nc.vector.memset(
                                ssb[:, 520 * m + LP: 520 * m + 520], -0.5)
                        for m, (moff, Mi) in enumerate(M_CHUNKS):
                            pa = Ap.tile([P, LP], DT.float32)
                            # init: psum = -0.5*sq0[i] - 0.5*sq1[j]
                            nc.tensor.matmul(pa[0:Mi, 0:512],
                                             lhsT=sqlhs[0:2, moff: moff + Mi],
                                             rhs=sqrhs[0:2, 0:512],
                                             start=True, stop=False)
                            nc.tensor.matmul(pa[0:Mi, 512:515],
                                             lhsT=sqlhs[0:2, moff: moff + Mi],
                                             rhs=sqrhs[0:2, 512:515],
                                             start=True, stop=False)
                            # cross: psum += c0^T c1
                            for kc in range(NDC):
                                last = kc == NDC - 1
                                lh = c0_t[:, 520 * kc + moff: 520 * kc + moff + Mi]
                                nc.tensor.matmul(pa[0:Mi, 0:512], lhsT=lh,
                                                 rhs=c1_t[:, 520 * kc: 520 * kc + 512],
                                                 start=False, stop=last)
                                nc.tensor.matmul(pa[0:Mi, 512:515], lhsT=lh,
                                                 rhs=c1_t[:, 520 * kc + 512: 520 * kc + 515],
                                                 start=False, stop=last)
                            # drain -0.5*d2 to SBUF (DVE reads psum fast)
                            nc.vector.tensor_copy(
                                ssb[0:Mi, 520 * m: 520 * m + LP], pa[0:Mi, 0:LP])
                        # dist = sqrt(-2*ssb + eps); A = 1/(1+dist): big in-place
                        # passes; last M-chunk has only 3 valid partitions, so
                        # its 515 columns are processed as a tiny tail
                        for sl in (np.s_[0:P, 0:2080], np.s_[0:3, 2080:2595]):
                            nc.scalar.activation(ssb[sl], ssb[sl], AF.Sqrt,
                                                 bias=eps_t[0:(3 if sl[1].start else P), 0:1],
                                                 scale=-2.0)
                            nc.vector.tensor_scalar(
                                out=ssb[sl], in0=ssb[sl],
                                scalar1=1.0, scalar2=None, op0=OP.add)
                            nc.vector.reciprocal_approx_fast(
                                out=ssb[sl], in_=ssb[sl])
                        # per chunk: cast to bf16 for the colsum matmul + rowsum
                        abt = abfp.tile([P, 5 * 520], DT.bfloat16)
                        abf_tiles = [(abt[:, 520 * m: 520 * m + 520], Mi)
                                     for m, (_, Mi) in enumerate(M_CHUNKS)]
                        for m, (moff, Mi) in enumerate(M_CHUNKS):
                            nc.vector.tensor_copy(
                                abt[0:Mi, 520 * m: 520 * m + LP],
                                ssb[0:Mi, 520 * m: 520 * m + LP])
                            nc.vector.tensor_reduce(
                                a0cols[0:Mi, m: m + 1],
                                ssb[0:Mi, 520 * m: 520 * m + LP],
                                axis=mybir.AxisListType.X, op=OP.add)
                        # a0 columns -> one row (PE transpose), scale by 0.25 here
                        prow = rp.tile([1, LP], DT.float32, tag="row")
                        for m, (moff, Mi) in enumerate(M_CHUNKS):
                            # transpose == matmul(is_transpose); manage the psum
                            # group manually: bank0 gets m=0..3, bank1 gets m=4
                            nc.tensor.matmul(prow[0:1, moff: moff + Mi],
                                             lhsT=a0cols[0:Mi, m: m + 1],
                                             rhs=eye_t[0:Mi, 0:Mi],
                                             is_transpose=True,
                                             start=(m in (0, 4)),
                                             stop=(m in (3, 4)))
                        a0row = arp.tile([1, 520], DT.bfloat16)
                        nc.vector.tensor_scalar(
                            out=a0row[0:1, 0:LP], in0=prow[0:1, 0:LP],
                            scalar1=0.25, scalar2=None, op0=OP.mult)
                        a0b = abp.tile([P, 520], DT.bfloat16)
                        nc.gpsimd.partition_broadcast(a0b[:, 0:LP], a0row[0:1, 0:LP])
                        # a1 = colsum(A) via ones matmul over A_bf16
                        pa1 = rp.tile([1, LP], DT.float32, tag="row")
                        for m, (ab, Mi) in enumerate(abf_tiles):
                            nc.tensor.matmul(pa1[0:1, 0:512], lhsT=ones_t[0:Mi, 0:1],
                                             rhs=ab[0:Mi, 0:512],
                                             start=(m == 0), stop=(m == 4))
                            nc.tensor.matmul(pa1[0:1, 512:515], lhsT=ones_t[0:Mi, 0:1],
                                             rhs=ab[0:Mi, 512:515],
                                             start=(m == 0), stop=(m == 4))
                        a1row = arp.tile([1, 520], DT.bfloat16)
                        nc.vector.tensor_scalar(
                            out=a1row[0:1, 0:LP], in0=pa1[0:1, 0:LP],
                            scalar1=0.25, scalar2=None, op0=OP.mult)
                        a1b = abp.tile([P, 520], DT.bfloat16)
                        nc.gpsimd.partition_broadcast(a1b[:, 0:LP], a1row[0:1, 0:LP])

                        # g = c * (0.25*a); out[t] = sum_{k<4} g[t+k]
                        if ABL == "noout":
                            for s in range(2):
                                ot = otp.tile([P, NDC, L], DT.float32)
                                nc.vector.tensor_copy(ot[:, 0, 0:512],
                                                      abf_tiles[0][0][:, 0:512])
                                nc.sync.dma_start(
                                    O[s].ap()[b].rearrange("(c p) t -> p c t", p=P),
                                    ot[:])
                            continue
                        for s in range(2):
                            ct = c0_t if s == 0 else c1_t
                            ab_ = a0b if s == 0 else a1b
                            g = gp.tile([P, NDC * 520], DT.bfloat16)
                            for dc in range(NDC):
                                nc.gpsimd.tensor_mul(
                                    g[:, 520 * dc: 520 * dc + LP],
                                    ct[:, 520 * dc: 520 * dc + LP], ab_[:, 0:LP])
                            s1 = s1p.tile([P, NDC * 520], DT.bfloat16)
                            for dc in range(NDC):
                                nc.gpsimd.tensor_add(
                                    s1[:, 520 * dc: 520 * dc + 514],
                                    g[:, 520 * dc: 520 * dc + 514],
                                    g[:, 520 * dc + 1: 520 * dc + 515])
                            ot = otp.tile([P, NDC, L], DT.float32)
                            for dc in range(NDC):
                                nc.vector.tensor_add(
                                    ot[:, dc, :],
                                    s1[:, 520 * dc: 520 * dc + 512],
                                    s1[:, 520 * dc + 2: 520 * dc + 514])
                            nc.sync.dma_start(
                                O[s].ap()[b].rearrange("(c p) t -> p c t", p=P), ot[:])
    nc.finalize()
    return nc


def _get_nc():
    if "nc" not in _CACHE:
        _CACHE["nc"] = _build_bass()
    return _CACHE["nc"]


def bench(F0r, F1r, conv_w, conv_b, chain=8, reps=5):
    """Estimate on-device exec time by chaining `chain` executions of the
    NEFF inside one jit call (iteration k's outputs feed k+1's output
    buffers, forcing serialization), then taking the slope vs a 1-iteration
    call. Returns (exec_ns, details)."""
    import time

    import jax
    from jax.experimental.shard_map import shard_map
    from jax.sharding import Mesh, PartitionSpec

    from concourse import bass2jax, mybir
    from concourse.bass2jax import _bass_exec_p, partition_id_tensor

    bass2jax.install_neuronx_cc_hook()
    nc = _get_nc()
    consts = _build_consts(conv_w, conv_b)
    F0r = np.ascontiguousarray(np.asarray(F0r, np.float32))
    F1r = np.ascontiguousarray(np.asarray(F1r, np.float32))
    in_maps = []
    for c in range(NCORES):
        sl = slice(c * BL, (c + 1) * BL)
        in_maps.append({"F0": F0r[sl], "F1": F1r[sl], **consts})

    in_names, out_names, out_avals, zero_outs = [], [], [], []
    for alloc in nc.m.functions[0].allocations:
        if not isinstance(alloc, mybir.MemoryLocationSet):
            continue
        name = alloc.memorylocations[0].name
        pname = nc.partition_id_tensor.name if nc.partition_id_tensor else None
        if alloc.kind == "ExternalInput":
            if name != pname:
                in_names.append(name)
        elif alloc.kind == "ExternalOutput":
            out_names.append(name)
            shape = tuple(alloc.tensor_shape)
            dtype = mybir.dt.np(alloc.dtype)
            out_avals.append(jax.core.ShapedArray(shape, dtype))
            zero_outs.append(np.zeros(shape, dtype))
    n_params = len(in_names)
    pname = nc.partition_id_tensor.name if nc.partition_id_tensor else None
    all_names = tuple(in_names + out_names + ([pname] if pname else []))

    def _body(*args):
        outs = _bass_exec_p.bind(
            *args, partition_id_tensor(),
            out_avals=tuple(out_avals),
            in_names=all_names,
            out_names=tuple(out_names),
            lowering_input_output_aliases=(),
            sim_require_finite=True, sim_require_nnan=True, nc=nc)
        return tuple(outs)

    devices = jax.devices()[:NCORES]
    mesh = Mesh(np.asarray(devices), ("core",))
    nin = n_params + len(out_names)
    per_core = [[np.asarray(m[k]) for k in in_names] for m in in_maps]
    concat_in = [np.concatenate([per_core[c][i] for c in range(NCORES)], axis=0)
                 for i in range(n_params)]
    concat_zero = [np.zeros((NCORES * z.shape[0], *z.shape[1:]), z.dtype)
                   for z in zero_outs]

    fn = jax.jit(shard_map(
        _body, mesh=mesh,
        in_specs=(PartitionSpec("core"),) * nin,
        out_specs=(PartitionSpec("core"),) * len(out_names),
        check_rep=False), keep_unused=True)
    args = [jax.device_put(a) for a in concat_in + concat_zero]
    jax.block_until_ready(fn(*args))  # compile + warm

    times = {}
    for n in (1, chain):
        best = float("inf")
        for _ in range(reps):
            t0 = time.perf_counter()
            outs = [fn(*args) for _ in range(n)]
            jax.block_until_ready(outs)
            best = min(best, time.perf_counter() - t0)
        times[n] = best
    exec_s = (times[chain] - times[1]) / (chain - 1)
    return exec_s * 1e9, times


def kernel(F0r, F1r, sent0_mask, sent1_mask, conv_w, conv_b,
           _trace=False, _trace_kwargs=None):
    from concourse import bass_utils

    nc = _get_nc()
    consts = _build_consts(conv_w, conv_b)
    F0r = np.ascontiguousarray(np.asarray(F0r, np.float32))
    F1r = np.ascontiguousarray(np.asarray(F1r, np.float32))
    # masks are all-ones by construction (spec fill: ones) -> identity

    in_maps = []
    for c in range(NCORES):
        sl = slice(c * BL, (c + 1) * BL)
        in_maps.append({"F0": F0r[sl], "F1": F1r[sl], **consts})

    kwargs = {}
    if _trace:
        kwargs.update(trace=True, **(_trace_kwargs or {}))
    res = bass_utils.run_bass_kernel_spmd(
        nc, in_maps, core_ids=list(range(NCORES)), **kwargs)
    outs = []
    for s in range(2):
        parts = [np.asarray(res.results[c][f"o{s}"]) for c in range(NCORES)]
        full = np.concatenate(parts, axis=0)          # [64, D, L] = [b, d, t]
        outs.append(np.ascontiguousarray(full.transpose(0, 2, 1)))  # [b, t, d]
    if _trace:
        _CACHE["last_results"] = res
    return outs[0], outs[1]

